# revision 1
# baseline (speedup 1.0000x reference)
"""ChebyKAN layer kernel for 8x Trainium2 NeuronCores.

Computes y[b,o] = sum_{i,d} T_d(tanh(x[b,i])) * C[i,o,d], d = 0..8,
with T_d the Chebyshev polynomials, via:
  - batch sharded 8 ways (1024 rows/core)
  - device computes T_1..T_8 with Chebyshev product identities
    (fp32 DVE/ACT), casts basis to bf16
  - d=0 term (T_0 == 1) folded into a host-precomputed bias[o]
  - big contraction as bf16 matmuls accumulating fp32 in PSUM:
    K = (i,d) of size 8192 in 64 chunks of 128
  - x is transposed on host so the basis is produced directly in
    [K, batch] (lhsT) layout; no on-device transpose needed.

Self-contained: hardcodes all shapes for inputs
  x: [8192, 1024] f32, cheby_coeffs: [1024, 1024, 9] f32.
"""

import numpy as np
import ml_dtypes

import concourse.bass as bass
import concourse.mybir as mybir
import concourse.tile as tile
from concourse import bacc
from concourse.bass_utils import run_bass_kernel_spmd

P = 128
B_TOTAL = 8192
I_DIM = 1024
O_DIM = 1024
DEG = 8              # degrees 1..8 on device (d=0 folded into bias)
N_CORES = 8
B_LOCAL = B_TOTAL // N_CORES     # 1024
IC = I_DIM // P                  # 8 input chunks
NK = IC * DEG                    # 64 K-chunks of 128
OH = 2                           # output halves (PSUM capacity: 8 banks)
ON = O_DIM // OH                 # 512

_nc = None
last_results = None  # BassKernelResults of the most recent run (for profiling)


def _ensure_ntff_hook():
    """bass_utils' trace path imports antenv.axon_hooks unconditionally, but
    this agent image's antenv package lacks that module. Synthesize it (with
    the real libaxon NTFF hook when available) so a BASS_TRACE=1 run traces
    instead of crashing."""
    import sys
    import types

    try:
        import antenv.axon_hooks  # noqa: F401
        return
    except ImportError:
        pass
    try:
        import antenv
    except ImportError:
        return
    hook = None
    try:
        from trn_agent_boot.trn_boot import _ntff_profile_via_ctypes
        hook = _ntff_profile_via_ctypes("/opt/axon/libaxon_pjrt.so")
    except Exception:
        hook = None
    mod = types.ModuleType("antenv.axon_hooks")
    state = {"hook": hook}
    mod.set_axon_ntff_profile_hook = lambda h: state.__setitem__("hook", h)
    mod.get_axon_ntff_profile_hook = lambda: state["hook"]
    sys.modules["antenv.axon_hooks"] = mod
    antenv.axon_hooks = mod


_ensure_ntff_hook()


def _build_nc():
    nc = bacc.Bacc()
    f32 = mybir.dt.float32
    bf16 = mybir.dt.bfloat16
    AF = mybir.ActivationFunctionType
    ALU = mybir.AluOpType

    xt_d = nc.dram_tensor("xt", [I_DIM, B_LOCAL], f32, kind="ExternalInput")
    w_d = nc.dram_tensor("w", [OH, NK, P, ON], bf16, kind="ExternalInput")
    bias_d = nc.dram_tensor("bias", [P, O_DIM], f32, kind="ExternalInput")
    y_d = nc.dram_tensor("y", [B_LOCAL, O_DIM], f32, kind="ExternalOutput")

    with tile.TileContext(nc) as tc:
        with (
            tc.tile_pool(name="const", bufs=1) as cpool,
            tc.tile_pool(name="xin", bufs=2) as xpool,
            tc.tile_pool(name="fwork", bufs=2) as fpool,
            tc.tile_pool(name="basis", bufs=1) as bpool,
            tc.tile_pool(name="wstream", bufs=12) as wpool,
            tc.tile_pool(name="outbuf", bufs=4) as opool,
            tc.tile_pool(name="acc", bufs=1, space="PSUM") as ppool,
        ):
            # ---- PE warm-up ----
            # HAM un-throttles the PE clock (1.2 -> 2.4 GHz) only after
            # ~3.4us of sustained matmul activity. Burn that window on dummy
            # matmuls into psum bank 0 while the first xt/wt DMAs are in
            # flight; the real k=0 matmul re-starts the bank (start=True).
            warm = cpool.tile([P, ON], bf16, name="warm")
            nc.vector.memset(warm, 1.0)
            warm_ps = ppool.tile([P, ON], f32, tag="ps0", name="warm_ps")
            for wi in range(8):
                nc.tensor.matmul(warm_ps, warm[:, 0:P], warm,
                                 start=(wi == 0), stop=(wi == 7))

            # ---- basis production: T_1..T_8 per 128-row chunk of i ----
            basis = {}

            for ic in range(IC):
                # ic == 0 runs every op on two half-tiles: the PE is already
                # warm when the kernel starts consuming, and half-granularity
                # lets the b<4 matmuls of each K-chunk start one half-op
                # earlier, which keeps the warm PE gapless during ramp-up.
                slices = ([slice(0, B_LOCAL // 2), slice(B_LOCAL // 2, B_LOCAL)]
                          if ic == 0 else [slice(0, B_LOCAL)])

                # xt on the HWDGE (sync) queue: issues in parallel with the
                # gpsimd wt stream and has lower first-byte latency.
                xt_t = xpool.tile([P, B_LOCAL], f32, tag="xt", name=f"xt_{ic}")
                for sl in slices:
                    nc.sync.dma_start(out=xt_t[:, sl],
                                      in_=xt_d[ic * P:(ic + 1) * P, sl])

                def btile(d):
                    bt = bpool.tile([P, B_LOCAL], bf16, tag=f"b_{ic}_{d}",
                                    name=f"b_{ic}_{d}")
                    basis[(ic, d)] = bt
                    return bt

                # T1 = tanh(x) (no clip: the recurrence is stable for |t|<=1
                # and T_d(+-1) is finite; deviation from the reference's
                # clip at 0.999 is ~1e-6 on y)
                t = fpool.tile([P, B_LOCAL], f32, tag="T1", name=f"t_{ic}")
                s2 = fpool.tile([P, B_LOCAL], f32, tag="sq", name=f"s2_{ic}")
                T2 = fpool.tile([P, B_LOCAL], f32, tag="T2", name=f"T2_{ic}",
                                bufs=1)
                V3 = fpool.tile([P, B_LOCAL], f32, tag="u", name=f"V3_{ic}")
                T3 = fpool.tile([P, B_LOCAL], f32, tag="T3", name=f"T3_{ic}",
                                bufs=1)
                s4 = fpool.tile([P, B_LOCAL], f32, tag="sq", name=f"s4_{ic}")
                T4 = fpool.tile([P, B_LOCAL], f32, tag="T4", name=f"T4_{ic}",
                                bufs=1)
                u5 = fpool.tile([P, B_LOCAL], bf16, tag="ub", name=f"u5_{ic}")
                s6 = fpool.tile([P, B_LOCAL], f32, tag="sq", name=f"s6_{ic}")
                u7 = fpool.tile([P, B_LOCAL], bf16, tag="ub", name=f"u7_{ic}")
                s8 = fpool.tile([P, B_LOCAL], f32, tag="sq", name=f"s8_{ic}")
                b1, b2, b3, b4 = btile(1), btile(2), btile(3), btile(4)
                b5, b6, b7, b8 = btile(5), btile(6), btile(7), btile(8)

                for sl in slices:
                    nc.scalar.activation(t[:, sl], xt_t[:, sl], AF.Tanh)
                    # DVE cast: shortens the tanh -> first-matmul chain
                    nc.vector.tensor_copy(b1[:, sl], t[:, sl])

                    # T2 = 2 t^2 - 1
                    nc.scalar.square(s2[:, sl], t[:, sl])
                    nc.vector.tensor_scalar(T2[:, sl], s2[:, sl], 2.0, -1.0,
                                            ALU.mult, ALU.add)
                    nc.scalar.copy(b2[:, sl], T2[:, sl])

                    # T3 = 2 t T2 - t = t * (2 T2 - 1)
                    nc.vector.tensor_scalar(V3[:, sl], T2[:, sl], 2.0, -1.0,
                                            ALU.mult, ALU.add)
                    nc.vector.tensor_mul(T3[:, sl], t[:, sl], V3[:, sl])
                    nc.scalar.copy(b3[:, sl], T3[:, sl])

                    # T4 = 2 T2^2 - 1
                    nc.scalar.square(s4[:, sl], T2[:, sl])
                    nc.vector.tensor_scalar(T4[:, sl], s4[:, sl], 2.0, -1.0,
                                            ALU.mult, ALU.add)
                    nc.scalar.copy(b4[:, sl], T4[:, sl])

                    # Degrees 5..8 are leaves (no downstream consumer), so
                    # they can be produced in cheaper precision/modes:
                    #   T5 = 2 T2 T3 - T1, T7 = 2 T3 T4 - T1 from bf16
                    #   operands (bf16 DVE ops run in 2x mode)
                    #   T6 = 2 T3^2 - 1, T8 = 2 T4^2 - 1 as one tensor_scalar
                    #   with direct bf16 output (fp32 squares from ACT)
                    nc.vector.tensor_mul(u5[:, sl], b2[:, sl], b3[:, sl])
                    nc.vector.scalar_tensor_tensor(b5[:, sl], u5[:, sl], 2.0,
                                                   b1[:, sl],
                                                   ALU.mult, ALU.subtract)

                    nc.scalar.square(s6[:, sl], T3[:, sl])
                    nc.vector.tensor_scalar(b6[:, sl], s6[:, sl], 2.0, -1.0,
                                            ALU.mult, ALU.add)

                    nc.vector.tensor_mul(u7[:, sl], b3[:, sl], b4[:, sl])
                    nc.vector.scalar_tensor_tensor(b7[:, sl], u7[:, sl], 2.0,
                                                   b1[:, sl],
                                                   ALU.mult, ALU.subtract)

                    nc.scalar.square(s8[:, sl], T4[:, sl])
                    nc.vector.tensor_scalar(b8[:, sl], s8[:, sl], 2.0, -1.0,
                                            ALU.mult, ALU.add)

            # bias is only consumed at the end of each o-half pass; load it
            # late so it doesn't delay the xt/wt streams.
            bias_t = cpool.tile([P, O_DIM], f32, name="bias_t")
            nc.sync.dma_start(out=bias_t, in_=bias_d[:, :])

            # ---- contraction: two o-half passes over all 64 K-chunks ----
            psums = [ppool.tile([P, ON], f32, tag=f"ps{b}", name=f"ps{b}")
                     for b in range(B_LOCAL // P)]
            # pass 0: o-half 0, all 8 batch banks (overlaps basis production)
            # pass 1a/1b: o-half 1 split in two bank halves, so the first
            # half's bias-adds + stores overlap the second half's matmuls
            # and the final tail only drains 4 banks.
            passes = [(0, 0, 8), (1, 0, 4), (1, 4, 8)]
            for pi, (oh, blo, bhi) in enumerate(passes):
                for k in range(NK):
                    ic, dm1 = divmod(k, DEG)
                    wt = wpool.tile([P, ON], bf16, tag="wt",
                                    name=f"wt_{pi}_{k}")
                    nc.gpsimd.dma_start(out=wt, in_=w_d[oh, k])
                    bt = basis[(ic, dm1 + 1)]
                    for b in range(blo, bhi):
                        nc.tensor.matmul(
                            psums[b],
                            bt[:, b * P:(b + 1) * P],
                            wt,
                            start=(k == 0),
                            stop=(k == NK - 1),
                        )
                for b in range(blo, bhi):
                    ot = opool.tile([P, ON], f32, tag="ot", name=f"ot_{pi}_{b}")
                    bias_sl = bias_t[:, oh * ON:(oh + 1) * ON]
                    if pi == 0 and b < 4:
                        # banks 0-3 gate pass 1a: drain them via ACT copy so
                        # the start=True matmuls aren't stuck behind the
                        # serial DVE bias-add chain; add bias in place later
                        # (overlaps the next pass).
                        nc.scalar.copy(ot, psums[b])
                        nc.vector.tensor_add(ot, ot, bias_sl)
                        nc.sync.dma_start(
                            out=y_d[b * P:(b + 1) * P,
                                    oh * ON:(oh + 1) * ON],
                            in_=ot)
                    elif pi == len(passes) - 1:
                        # final pass: half-granularity add+store so the DMA
                        # of the first half overlaps the second half's add
                        for hh in range(2):
                            hsl = slice(hh * (ON // 2), (hh + 1) * (ON // 2))
                            nc.vector.tensor_add(ot[:, hsl], psums[b][:, hsl],
                                                 bias_sl[:, hsl])
                            nc.sync.dma_start(
                                out=y_d[b * P:(b + 1) * P,
                                        oh * ON + hh * (ON // 2):
                                        oh * ON + (hh + 1) * (ON // 2)],
                                in_=ot[:, hsl])
                    else:
                        nc.vector.tensor_add(ot, psums[b], bias_sl)
                        nc.sync.dma_start(
                            out=y_d[b * P:(b + 1) * P,
                                    oh * ON:(oh + 1) * ON],
                            in_=ot)
    nc.compile()  # bacc legalization: splits multi-sem waits (TRN2 allows 1)
    return nc


def _get_nc():
    global _nc
    if _nc is None:
        _nc = _build_nc()
    return _nc


def _prep_inputs(x, cheby_coeffs):
    x = np.asarray(x, dtype=np.float32)
    C = np.asarray(cheby_coeffs, dtype=np.float32)
    bf16 = ml_dtypes.bfloat16

    # W[oh, k=(ic,d), p, on] = C[ic*128+p, oh*512+on, d+1]
    Wd = C[:, :, 1:]                                   # [I, O, 8]
    Wd = Wd.reshape(IC, P, OH, ON, DEG)                # [ic, p, oh, on, d]
    Wd = np.transpose(Wd, (2, 0, 4, 1, 3))             # [oh, ic, d, p, on]
    Wd = np.ascontiguousarray(Wd.reshape(OH, NK, P, ON)).astype(bf16)

    bias = C[:, :, 0].sum(axis=0, dtype=np.float64).astype(np.float32)
    bias_rep = np.ascontiguousarray(np.broadcast_to(bias, (P, O_DIM)))

    in_maps = []
    for c in range(N_CORES):
        xt = np.ascontiguousarray(x[c * B_LOCAL:(c + 1) * B_LOCAL, :].T)
        in_maps.append({"xt": xt, "w": Wd, "bias": bias_rep})
    return in_maps


def kernel(x, cheby_coeffs):
    global last_results
    nc = _get_nc()
    in_maps = _prep_inputs(x, cheby_coeffs)
    last_results = run_bass_kernel_spmd(nc, in_maps,
                                        core_ids=list(range(N_CORES)))
    y = np.concatenate([r["y"] for r in last_results.results], axis=0)
    return y



# revision 2
# speedup vs baseline: 1.1247x; 1.1247x over previous
"""ChebyKAN layer kernel for 8x Trainium2 NeuronCores.

Computes y[b,o] = sum_{i,d} T_d(tanh(x[b,i])) * C[i,o,d], d = 0..8,
with T_d the Chebyshev polynomials, via:
  - batch sharded 8 ways (1024 rows/core)
  - device computes T_1..T_8 with Chebyshev product identities
    (fp32 DVE/ACT)
  - d=0 term (T_0 == 1) folded into a host-precomputed bias[o]
  - mixed-precision contraction accumulating fp32 in PSUM:
      * most K-chunks as bf16 matmuls (128 K each)
      * FP8_PAIRS K-chunk pairs as fp8e4m3 DoubleRow matmuls (256 K
        per matmul at the same 512-cycle issue rate -> 2x throughput);
        the fp8 fraction is capped by the rel-err budget (2e-2)
  - all W pre-scaled by 2^18 on host so fp8 values use the e4m3 range;
    the output path rescales by 2^-18 and adds the bias in one DVE op
  - x is transposed on host so the basis is produced directly in
    [K, batch] (lhsT) layout; no on-device transpose needed.

Self-contained: hardcodes all shapes for inputs
  x: [8192, 1024] f32, cheby_coeffs: [1024, 1024, 9] f32.
"""

import numpy as np
import ml_dtypes

import concourse.bass as bass
import concourse.mybir as mybir
import concourse.tile as tile
from concourse import bacc
from concourse.bass_utils import run_bass_kernel_spmd

P = 128
B_TOTAL = 8192
I_DIM = 1024
O_DIM = 1024
DEG = 8              # degrees 1..8 on device (d=0 folded into bias)
N_CORES = 8
B_LOCAL = B_TOTAL // N_CORES     # 1024
IC = I_DIM // P                  # 8 input chunks
OH = 2                           # output halves (PSUM capacity: 8 banks)
ON = O_DIM // OH                 # 512

# fp8 DoubleRow assignment: per input-chunk, degree pairs computed in
# fp8e4m3 (both basis and W). Everything else is bf16.  20 of 64
# K-chunks in fp8 keeps max rel err ~0.017 (budget 2e-2).
FP8_PAIRS = {ic: [(7, 8)] for ic in range(IC)}
FP8_PAIRS[0] = [(5, 6), (7, 8)]
FP8_PAIRS[1] = [(5, 6), (7, 8)]

W_SCALE = float(2 ** 18)         # host multiplies W; device rescales
W_CLIP = 224.0                   # keep e4m3 (TRN: max 240) finite

_nc = None
last_results = None  # BassKernelResults of the most recent run (for profiling)


def _fp8_degs(ic):
    return {d for pr in FP8_PAIRS[ic] for d in pr}


def _chunk_lists():
    """bf16 chunk list [(ic, d)] and fp8 pair list [(ic, (dlo, dhi))],
    both in production order (ic-major, degree-minor)."""
    bf, f8 = [], []
    for ic in range(IC):
        degs = _fp8_degs(ic)
        for d in range(1, DEG + 1):
            if d not in degs:
                bf.append((ic, d))
        for pr in FP8_PAIRS[ic]:
            f8.append((ic, pr))
    return bf, f8


BF_CHUNKS, F8_PAIRLIST = _chunk_lists()
NB = len(BF_CHUNKS)        # 44
NF = len(F8_PAIRLIST)      # 10


def _ensure_ntff_hook():
    """bass_utils' trace path imports antenv.axon_hooks unconditionally, but
    this agent image's antenv package lacks that module. Synthesize it (with
    the real libaxon NTFF hook when available) so a BASS_TRACE=1 run traces
    instead of crashing."""
    import sys
    import types

    try:
        import antenv.axon_hooks  # noqa: F401
        return
    except ImportError:
        pass
    try:
        import antenv
    except ImportError:
        return
    hook = None
    try:
        from trn_agent_boot.trn_boot import _ntff_profile_via_ctypes
        hook = _ntff_profile_via_ctypes("/opt/axon/libaxon_pjrt.so")
    except Exception:
        hook = None
    mod = types.ModuleType("antenv.axon_hooks")
    state = {"hook": hook}
    mod.set_axon_ntff_profile_hook = lambda h: state.__setitem__("hook", h)
    mod.get_axon_ntff_profile_hook = lambda: state["hook"]
    sys.modules["antenv.axon_hooks"] = mod
    antenv.axon_hooks = mod


_ensure_ntff_hook()


def _build_nc():
    nc = bacc.Bacc()
    f32 = mybir.dt.float32
    bf16 = mybir.dt.bfloat16
    fp8 = mybir.dt.float8e4
    AF = mybir.ActivationFunctionType
    ALU = mybir.AluOpType
    DR = mybir.MatmulPerfMode.DoubleRow

    xt_d = nc.dram_tensor("xt", [I_DIM, B_LOCAL], f32, kind="ExternalInput")
    wb_d = nc.dram_tensor("wb", [OH, NB, P, ON], bf16, kind="ExternalInput")
    wf_d = nc.dram_tensor("wf", [OH, NF, P, 2, ON], fp8, kind="ExternalInput")
    bias_d = nc.dram_tensor("bias", [P, O_DIM], f32, kind="ExternalInput")
    y_d = nc.dram_tensor("y", [B_LOCAL, O_DIM], f32, kind="ExternalOutput")

    with tile.TileContext(nc) as tc:
        with (
            tc.tile_pool(name="const", bufs=1) as cpool,
            tc.tile_pool(name="xin", bufs=2) as xpool,
            tc.tile_pool(name="fwork", bufs=2) as fpool,
            tc.tile_pool(name="basis", bufs=1) as bpool,
            tc.tile_pool(name="wstream", bufs=12) as wpool,
            tc.tile_pool(name="outbuf", bufs=4) as opool,
            tc.tile_pool(name="acc", bufs=1, space="PSUM") as ppool,
        ):
            # ---- PE warm-up ----
            # HAM un-throttles the PE clock (1.2 -> 2.4 GHz) only after
            # ~3.4us of sustained matmul activity. Burn that window on dummy
            # matmuls into psum bank 0 while the first xt/wt DMAs are in
            # flight; the real k=0 matmul re-starts the bank (start=True).
            warm = cpool.tile([P, ON], bf16, name="warm")
            nc.vector.memset(warm, 1.0)
            warm_ps = ppool.tile([P, ON], f32, tag="ps0", name="warm_ps")
            for wi in range(10):
                nc.tensor.matmul(warm_ps, warm[:, 0:P], warm,
                                 start=(wi == 0), stop=(wi == 9))

            # ---- basis production: T_1..T_8 per 128-row chunk of i ----
            basis = {}       # (ic, d) -> bf16 tile [P, B_LOCAL]
            basis8 = {}      # (ic, (dlo, dhi)) -> fp8 pair tile [P, 2, B_LOCAL]

            for ic in range(IC):
                # ic == 0 runs at quarter granularity so the first real
                # matmuls can start ~1.5us in (PE warm + early stream);
                # everything else at full tile granularity.
                if ic == 0:
                    qs = B_LOCAL // 4
                    slices = [slice(i * qs, (i + 1) * qs) for i in range(4)]
                else:
                    slices = [slice(0, B_LOCAL)]

                degs8 = _fp8_degs(ic)

                # xt on the HWDGE (sync) queue: issues in parallel with the
                # gpsimd wt stream and has lower first-byte latency.
                xt_t = xpool.tile([P, B_LOCAL], f32, tag="xt", name=f"xt_{ic}")
                for sl in slices:
                    nc.sync.dma_start(out=xt_t[:, sl],
                                      in_=xt_d[ic * P:(ic + 1) * P, sl])

                def btile(d):
                    bt = bpool.tile([P, B_LOCAL], bf16, tag=f"b_{ic}_{d}",
                                    name=f"b_{ic}_{d}")
                    basis[(ic, d)] = bt
                    return bt

                def bdst(d):
                    """Output AP for degree d: slot of an fp8 pair tile if
                    d is fp8, else a bf16 basis tile."""
                    if d in degs8:
                        pr = next(p for p in FP8_PAIRS[ic] if d in p)
                        pt = basis8.get((ic, pr))
                        if pt is None:
                            pt = bpool.tile([P, 2, B_LOCAL], fp8,
                                            tag=f"bp_{ic}_{pr[0]}",
                                            name=f"bp_{ic}_{pr[0]}")
                            basis8[(ic, pr)] = pt
                        return pt[:, pr.index(d), :]
                    return btile(d)

                # T1 = tanh(x) (no clip: the recurrence is stable for |t|<=1
                # and T_d(+-1) is finite; deviation from the reference's
                # clip at 0.999 is ~1e-6 on y)
                t = fpool.tile([P, B_LOCAL], f32, tag="T1", name=f"t_{ic}")
                s2 = fpool.tile([P, B_LOCAL], f32, tag="sq", name=f"s2_{ic}")
                T2 = fpool.tile([P, B_LOCAL], f32, tag="T2", name=f"T2_{ic}",
                                bufs=1)
                V3 = fpool.tile([P, B_LOCAL], f32, tag="u", name=f"V3_{ic}")
                T3 = fpool.tile([P, B_LOCAL], f32, tag="T3", name=f"T3_{ic}",
                                bufs=1)
                s4 = fpool.tile([P, B_LOCAL], f32, tag="sq", name=f"s4_{ic}")
                T4 = fpool.tile([P, B_LOCAL], f32, tag="T4", name=f"T4_{ic}",
                                bufs=1)
                u5 = fpool.tile([P, B_LOCAL], bf16, tag="ub", name=f"u5_{ic}")
                s6 = fpool.tile([P, B_LOCAL], f32, tag="sq", name=f"s6_{ic}")
                u7 = fpool.tile([P, B_LOCAL], bf16, tag="ub", name=f"u7_{ic}")
                s8 = fpool.tile([P, B_LOCAL], f32, tag="sq", name=f"s8_{ic}")
                b1 = bdst(1)
                b2 = bdst(2)
                b3 = bdst(3)
                b4 = bdst(4)
                b5 = bdst(5)
                b6 = bdst(6)
                b7 = bdst(7)
                b8 = bdst(8)

                for sl in slices:
                    nc.scalar.activation(t[:, sl], xt_t[:, sl], AF.Tanh)
                    # DVE cast: shortens the tanh -> first-matmul chain
                    nc.vector.tensor_copy(b1[:, sl], t[:, sl])

                    # T2 = 2 t^2 - 1
                    nc.scalar.square(s2[:, sl], t[:, sl])
                    nc.vector.tensor_scalar(T2[:, sl], s2[:, sl], 2.0, -1.0,
                                            ALU.mult, ALU.add)
                    nc.scalar.copy(b2[:, sl], T2[:, sl])

                    # T3 = 2 t T2 - t = t * (2 T2 - 1)
                    nc.vector.tensor_scalar(V3[:, sl], T2[:, sl], 2.0, -1.0,
                                            ALU.mult, ALU.add)
                    nc.vector.tensor_mul(T3[:, sl], t[:, sl], V3[:, sl])
                    nc.scalar.copy(b3[:, sl], T3[:, sl])

                    # T4 = 2 T2^2 - 1
                    nc.scalar.square(s4[:, sl], T2[:, sl])
                    nc.vector.tensor_scalar(T4[:, sl], s4[:, sl], 2.0, -1.0,
                                            ALU.mult, ALU.add)
                    nc.scalar.copy(b4[:, sl], T4[:, sl])

                    # Degrees 5..8 are leaves (no downstream consumer), so
                    # they can be produced directly in their matmul dtype:
                    #   T5 = 2 T2 T3 - T1, T7 = 2 T3 T4 - T1 from bf16
                    #   operands (bf16 DVE ops run in 2x mode)
                    #   T6 = 2 T3^2 - 1, T8 = 2 T4^2 - 1 as one tensor_scalar
                    #   (fp32 squares from ACT)
                    nc.vector.tensor_mul(u5[:, sl], b2[:, sl], b3[:, sl])
                    nc.vector.scalar_tensor_tensor(b5[:, sl], u5[:, sl], 2.0,
                                                   b1[:, sl],
                                                   ALU.mult, ALU.subtract)

                    nc.scalar.square(s6[:, sl], T3[:, sl])
                    nc.vector.tensor_scalar(b6[:, sl], s6[:, sl], 2.0, -1.0,
                                            ALU.mult, ALU.add)

                    nc.vector.tensor_mul(u7[:, sl], b3[:, sl], b4[:, sl])
                    nc.vector.scalar_tensor_tensor(b7[:, sl], u7[:, sl], 2.0,
                                                   b1[:, sl],
                                                   ALU.mult, ALU.subtract)

                    nc.scalar.square(s8[:, sl], T4[:, sl])
                    nc.vector.tensor_scalar(b8[:, sl], s8[:, sl], 2.0, -1.0,
                                            ALU.mult, ALU.add)

            # bias is only consumed at the end of each o-half pass; load it
            # late so it doesn't delay the xt/wt streams.
            bias_t = cpool.tile([P, O_DIM], f32, name="bias_t")
            nc.sync.dma_start(out=bias_t, in_=bias_d[:, :])

            # ---- contraction: two o-half passes over all K-chunks ----
            psums = [ppool.tile([P, ON], f32, tag=f"ps{b}", name=f"ps{b}")
                     for b in range(B_LOCAL // P)]
            # pass 0: o-half 0, all 8 batch banks (overlaps basis production)
            # pass 1a/1b: o-half 1 split in two bank halves, so the first
            # half's bias-adds + stores overlap the second half's matmuls
            # and the final tail only drains 4 banks.
            passes = [(0, 0, 8), (1, 0, 4), (1, 4, 8)]
            RS = 1.0 / W_SCALE

            # per-ic chunk schedules (production order)
            ic_sched = []
            for ic in range(IC):
                degs8 = _fp8_degs(ic)
                bf_ds = [d for d in range(1, DEG + 1) if d not in degs8]
                ic_sched.append((ic, bf_ds, FP8_PAIRS[ic]))

            def bf_index(ic, d):
                return BF_CHUNKS.index((ic, d))

            def f8_index(ic, pr):
                return F8_PAIRLIST.index((ic, pr))

            for pi, (oh, blo, bhi) in enumerate(passes):
                first = True
                for ic, bf_ds, prs in ic_sched:
                    for d in bf_ds:
                        kb = bf_index(ic, d)
                        wt = wpool.tile([P, ON], bf16, tag="wt",
                                        name=f"wt_{pi}_{kb}")
                        nc.gpsimd.dma_start(out=wt, in_=wb_d[oh, kb])
                        bt = basis[(ic, d)]
                        last = (ic == IC - 1 and d == bf_ds[-1]
                                and not prs)
                        for b in range(blo, bhi):
                            nc.tensor.matmul(
                                psums[b],
                                bt[:, b * P:(b + 1) * P],
                                wt,
                                start=first,
                                stop=last,
                            )
                        first = False
                    for pr in prs:
                        kf = f8_index(ic, pr)
                        wt8 = wpool.tile([P, 2, ON], fp8, tag="wt8",
                                         name=f"wt8_{pi}_{kf}")
                        nc.gpsimd.dma_start(out=wt8, in_=wf_d[oh, kf])
                        pt = basis8[(ic, pr)]
                        last = (ic == IC - 1 and pr == prs[-1])
                        for b in range(blo, bhi):
                            nc.tensor.matmul(
                                psums[b],
                                pt[:, :, b * P:(b + 1) * P],
                                wt8,
                                start=first,
                                stop=last,
                                perf_mode=DR,
                            )
                        first = False

                for b in range(blo, bhi):
                    ot = opool.tile([P, ON], f32, tag="ot", name=f"ot_{pi}_{b}")
                    bias_sl = bias_t[:, oh * ON:(oh + 1) * ON]
                    if pi == 0 and b < 4:
                        # banks 0-3 gate pass 1a: drain them via ACT copy so
                        # the start=True matmuls aren't stuck behind the
                        # serial DVE chain; rescale+bias in place later
                        # (overlaps the next pass).
                        nc.scalar.copy(ot, psums[b])
                        nc.vector.scalar_tensor_tensor(ot, ot, RS, bias_sl,
                                                       ALU.mult, ALU.add)
                        nc.sync.dma_start(
                            out=y_d[b * P:(b + 1) * P,
                                    oh * ON:(oh + 1) * ON],
                            in_=ot)
                    elif pi == len(passes) - 1:
                        # final pass: half-granularity rescale+bias, stores
                        # split across both DMA queues (the gpsimd queue is
                        # done with W by now) so the drain isn't serialized
                        # on one queue.
                        for hh in range(2):
                            hsl = slice(hh * (ON // 2), (hh + 1) * (ON // 2))
                            nc.vector.scalar_tensor_tensor(
                                ot[:, hsl], psums[b][:, hsl], RS,
                                bias_sl[:, hsl], ALU.mult, ALU.add)
                            q = nc.sync if (b % 2 == 0) else nc.gpsimd
                            q.dma_start(
                                out=y_d[b * P:(b + 1) * P,
                                        oh * ON + hh * (ON // 2):
                                        oh * ON + (hh + 1) * (ON // 2)],
                                in_=ot[:, hsl])
                    else:
                        nc.vector.scalar_tensor_tensor(ot, psums[b], RS,
                                                       bias_sl,
                                                       ALU.mult, ALU.add)
                        nc.sync.dma_start(
                            out=y_d[b * P:(b + 1) * P,
                                    oh * ON:(oh + 1) * ON],
                            in_=ot)
    nc.compile()  # bacc legalization: splits multi-sem waits (TRN2 allows 1)
    return nc


def _get_nc():
    global _nc
    if _nc is None:
        _nc = _build_nc()
    return _nc


def _prep_inputs(x, cheby_coeffs):
    x = np.asarray(x, dtype=np.float32)
    C = np.asarray(cheby_coeffs, dtype=np.float32)
    bf16 = ml_dtypes.bfloat16
    e4 = ml_dtypes.float8_e4m3

    Ws = C[:, :, 1:] * np.float32(W_SCALE)             # [I, O, 8] scaled
    Ws = Ws.reshape(IC, P, OH, ON, DEG)                # [ic, p, oh, on, d]

    # bf16 chunks: Wb[oh, kb, p, on]
    Wb = np.empty((OH, NB, P, ON), dtype=bf16)
    for kb, (ic, d) in enumerate(BF_CHUNKS):
        Wb[:, kb] = np.transpose(Ws[ic, :, :, :, d - 1],
                                 (1, 0, 2)).astype(bf16)

    # fp8 pairs: Wf[oh, kf, p, 2, on]
    Wf = np.empty((OH, NF, P, 2, ON), dtype=e4)
    for kf, (ic, pr) in enumerate(F8_PAIRLIST):
        for si, d in enumerate(pr):
            w = np.clip(Ws[ic, :, :, :, d - 1], -W_CLIP, W_CLIP)
            Wf[:, kf, :, si, :] = np.transpose(w, (1, 0, 2)).astype(e4)

    bias = C[:, :, 0].sum(axis=0, dtype=np.float64).astype(np.float32)
    bias_rep = np.ascontiguousarray(np.broadcast_to(bias, (P, O_DIM)))

    in_maps = []
    for c in range(N_CORES):
        xt = np.ascontiguousarray(x[c * B_LOCAL:(c + 1) * B_LOCAL, :].T)
        in_maps.append({"xt": xt, "wb": Wb, "wf": Wf, "bias": bias_rep})
    return in_maps


def kernel(x, cheby_coeffs):
    global last_results
    nc = _get_nc()
    in_maps = _prep_inputs(x, cheby_coeffs)
    last_results = run_bass_kernel_spmd(nc, in_maps,
                                        core_ids=list(range(N_CORES)))
    y = np.concatenate([r["y"] for r in last_results.results], axis=0)
    return y


# revision 11
# speedup vs baseline: 1.1322x; 1.0066x over previous
"""ChebyKAN layer kernel for 8x Trainium2 NeuronCores.

Computes y[b,o] = sum_{i,d} T_d(tanh(x[b,i])) * C[i,o,d], d = 0..8,
with T_d the Chebyshev polynomials, via:
  - batch sharded 8 ways (1024 rows/core)
  - device computes T_1..T_8 with Chebyshev product identities
    (fp32 DVE/ACT)
  - d=0 term (T_0 == 1) folded into a host-precomputed bias[o]
  - mixed-precision contraction accumulating fp32 in PSUM:
      * most K-chunks as bf16 matmuls (128 K each)
      * FP8_PAIRS K-chunk pairs as fp8e4m3 DoubleRow matmuls (256 K
        per matmul at the same 512-cycle issue rate -> 2x throughput);
        the fp8 fraction is capped by the rel-err budget (2e-2)
  - all W pre-scaled by 2^18 on host so fp8 values use the e4m3 range;
    the output path rescales by 2^-18 and adds the bias in one DVE op
  - x is transposed on host so the basis is produced directly in
    [K, batch] (lhsT) layout; no on-device transpose needed.

Self-contained: hardcodes all shapes for inputs
  x: [8192, 1024] f32, cheby_coeffs: [1024, 1024, 9] f32.
"""

import numpy as np
import ml_dtypes

import concourse.bass as bass
import concourse.mybir as mybir
import concourse.tile as tile
from concourse import bacc
from concourse.bass_utils import run_bass_kernel_spmd

P = 128
B_TOTAL = 8192
I_DIM = 1024
O_DIM = 1024
DEG = 8              # degrees 1..8 on device (d=0 folded into bias)
N_CORES = 8
B_LOCAL = B_TOTAL // N_CORES     # 1024
IC = I_DIM // P                  # 8 input chunks
OH = 2                           # output halves (PSUM capacity: 8 banks)
ON = O_DIM // OH                 # 512

# fp8 DoubleRow assignment: per input-chunk, degree pairs computed in
# fp8e4m3 (both basis and W). Everything else is bf16.  22 of 64
# K-chunks in fp8 keeps max rel err ~0.018 (budget 2e-2).
FP8_PAIRS = {ic: [(7, 8)] for ic in range(IC)}
FP8_PAIRS[0] = [(5, 6), (7, 8)]
FP8_PAIRS[1] = [(5, 6), (7, 8)]
FP8_PAIRS[2] = [(5, 6), (7, 8)]

W_SCALE = float(2 ** 18)         # host multiplies W; device rescales
W_CLIP = 224.0                   # keep e4m3 (TRN: max 240) finite

_nc = None
last_results = None  # BassKernelResults of the most recent run (for profiling)


def _fp8_degs(ic):
    return {d for pr in FP8_PAIRS[ic] for d in pr}


def _chunk_lists():
    """bf16 chunk-pair list [(ic, (da, db))] and fp8 pair list
    [(ic, (dlo, dhi))], both in production order (ic-major,
    degree-minor). Every ic has an even number of bf16 degrees, so
    bf16 chunks pair up within an ic (one DMA per pair)."""
    bf, f8 = [], []
    for ic in range(IC):
        degs = _fp8_degs(ic)
        ds = [d for d in range(1, DEG + 1) if d not in degs]
        assert len(ds) % 2 == 0
        for j in range(0, len(ds), 2):
            bf.append((ic, (ds[j], ds[j + 1])))
        for pr in FP8_PAIRS[ic]:
            f8.append((ic, pr))
    return bf, f8


BF_PAIRLIST, F8_PAIRLIST = _chunk_lists()
NBP = len(BF_PAIRLIST)     # 21 with n=22
NF = len(F8_PAIRLIST)      # 11


def _ensure_ntff_hook():
    """bass_utils' trace path imports antenv.axon_hooks unconditionally, but
    this agent image's antenv package lacks that module. Synthesize it (with
    the real libaxon NTFF hook when available) so a BASS_TRACE=1 run traces
    instead of crashing."""
    import sys
    import types

    try:
        import antenv.axon_hooks  # noqa: F401
        return
    except ImportError:
        pass
    try:
        import antenv
    except ImportError:
        return
    hook = None
    try:
        from trn_agent_boot.trn_boot import _ntff_profile_via_ctypes
        hook = _ntff_profile_via_ctypes("/opt/axon/libaxon_pjrt.so")
    except Exception:
        hook = None
    mod = types.ModuleType("antenv.axon_hooks")
    state = {"hook": hook}
    mod.set_axon_ntff_profile_hook = lambda h: state.__setitem__("hook", h)
    mod.get_axon_ntff_profile_hook = lambda: state["hook"]
    sys.modules["antenv.axon_hooks"] = mod
    antenv.axon_hooks = mod


_ensure_ntff_hook()


def _build_nc():
    nc = bacc.Bacc()
    f32 = mybir.dt.float32
    bf16 = mybir.dt.bfloat16
    fp8 = mybir.dt.float8e4
    AF = mybir.ActivationFunctionType
    ALU = mybir.AluOpType
    DR = mybir.MatmulPerfMode.DoubleRow

    xt_d = nc.dram_tensor("xt", [I_DIM, B_LOCAL], f32, kind="ExternalInput")
    wb_d = nc.dram_tensor("wb", [OH, NBP, P, 2, ON], bf16, kind="ExternalInput")
    wf_d = nc.dram_tensor("wf", [OH, NF, P, 2, ON], fp8, kind="ExternalInput")
    bias_d = nc.dram_tensor("bias", [P, O_DIM], f32, kind="ExternalInput")
    y_d = nc.dram_tensor("y", [B_LOCAL, O_DIM], f32, kind="ExternalOutput")

    with tile.TileContext(nc) as tc:
        with (
            tc.tile_pool(name="const", bufs=1) as cpool,
            tc.tile_pool(name="xin", bufs=2) as xpool,
            tc.tile_pool(name="fwork", bufs=2) as fpool,
            tc.tile_pool(name="basis", bufs=1) as bpool,
            tc.tile_pool(name="wstream", bufs=12) as wpool,
            tc.tile_pool(name="outbuf", bufs=4) as opool,
            tc.tile_pool(name="acc", bufs=1, space="PSUM") as ppool,
        ):
            # ---- PE warm-up ----
            # HAM un-throttles the PE clock (1.2 -> 2.4 GHz) only after
            # ~3.4us of sustained matmul activity. Burn that window on dummy
            # matmuls into psum bank 0 while the first xt/wt DMAs are in
            # flight; the real k=0 matmul re-starts the bank (start=True).
            warm = cpool.tile([P, ON], bf16, name="warm")
            nc.vector.memset(warm, 1.0)
            warm_ps = ppool.tile([P, ON], f32, tag="ps0", name="warm_ps")
            for wi in range(3):
                nc.tensor.matmul(warm_ps, warm[:, 0:P], warm,
                                 start=(wi == 0), stop=(wi == 2))

            # ---- basis production: T_1..T_8 per 128-row chunk of i ----
            basis = {}       # (ic, d) -> bf16 tile [P, B_LOCAL]
            basis8 = {}      # (ic, (dlo, dhi)) -> fp8 pair tile [P, 2, B_LOCAL]

            for ic in range(IC):
                # ic == 0 runs every op on two half-tiles: the first real
                # matmuls can start one half-op earlier while the PE is
                # fresh off the warm-up.
                slices = ([slice(0, B_LOCAL // 2), slice(B_LOCAL // 2, B_LOCAL)]
                          if ic == 0 else [slice(0, B_LOCAL)])

                degs8 = _fp8_degs(ic)

                # xt on the HWDGE (sync) queue: issues in parallel with the
                # gpsimd wt stream and has lower first-byte latency.
                xt_t = xpool.tile([P, B_LOCAL], f32, tag="xt", name=f"xt_{ic}")
                for sl in slices:
                    nc.sync.dma_start(out=xt_t[:, sl],
                                      in_=xt_d[ic * P:(ic + 1) * P, sl])

                def btile(d):
                    bt = bpool.tile([P, B_LOCAL], bf16, tag=f"b_{ic}_{d}",
                                    name=f"b_{ic}_{d}")
                    basis[(ic, d)] = bt
                    return bt

                def bdst(d):
                    """Output AP for degree d: slot of an fp8 pair tile if
                    d is fp8, else a bf16 basis tile."""
                    if d in degs8:
                        pr = next(p for p in FP8_PAIRS[ic] if d in p)
                        pt = basis8.get((ic, pr))
                        if pt is None:
                            pt = bpool.tile([P, 2, B_LOCAL], fp8,
                                            tag=f"bp_{ic}_{pr[0]}",
                                            name=f"bp_{ic}_{pr[0]}")
                            basis8[(ic, pr)] = pt
                        return pt[:, pr.index(d), :]
                    return btile(d)

                # T1 = tanh(x) (no clip: the recurrence is stable for |t|<=1
                # and T_d(+-1) is finite; deviation from the reference's
                # clip at 0.999 is ~1e-6 on y)
                t = fpool.tile([P, B_LOCAL], f32, tag="T1", name=f"t_{ic}")
                s2 = fpool.tile([P, B_LOCAL], f32, tag="sq", name=f"s2_{ic}")
                T2 = fpool.tile([P, B_LOCAL], f32, tag="T2", name=f"T2_{ic}",
                                bufs=1)
                V3 = fpool.tile([P, B_LOCAL], f32, tag="u", name=f"V3_{ic}")
                T3 = fpool.tile([P, B_LOCAL], f32, tag="T3", name=f"T3_{ic}",
                                bufs=1)
                s4 = fpool.tile([P, B_LOCAL], f32, tag="sq", name=f"s4_{ic}")
                T4 = fpool.tile([P, B_LOCAL], f32, tag="T4", name=f"T4_{ic}",
                                bufs=1)
                u5 = fpool.tile([P, B_LOCAL], bf16, tag="ub", name=f"u5_{ic}")
                s6 = fpool.tile([P, B_LOCAL], f32, tag="sq", name=f"s6_{ic}")
                u7 = fpool.tile([P, B_LOCAL], bf16, tag="ub", name=f"u7_{ic}")
                s8 = fpool.tile([P, B_LOCAL], f32, tag="sq", name=f"s8_{ic}")
                b1 = bdst(1)
                b2 = bdst(2)
                b3 = bdst(3)
                b4 = bdst(4)
                b5 = bdst(5)
                b6 = bdst(6)
                b7 = bdst(7)
                b8 = bdst(8)

                for sl in slices:
                    nc.scalar.activation(t[:, sl], xt_t[:, sl], AF.Tanh)
                    # DVE cast: shortens the tanh -> first-matmul chain
                    nc.vector.tensor_copy(b1[:, sl], t[:, sl])

                    # T2 = 2 t^2 - 1
                    nc.scalar.square(s2[:, sl], t[:, sl])
                    nc.vector.tensor_scalar(T2[:, sl], s2[:, sl], 2.0, -1.0,
                                            ALU.mult, ALU.add)
                    nc.scalar.copy(b2[:, sl], T2[:, sl])

                    # T3 = 2 t T2 - t = t * (2 T2 - 1)
                    nc.vector.tensor_scalar(V3[:, sl], T2[:, sl], 2.0, -1.0,
                                            ALU.mult, ALU.add)
                    nc.vector.tensor_mul(T3[:, sl], t[:, sl], V3[:, sl])
                    # b3/b4 casts on DVE: the ACT engine is the busier one
                    # in the production phase
                    nc.vector.tensor_copy(b3[:, sl], T3[:, sl])

                    # T4 = 2 T2^2 - 1
                    nc.scalar.square(s4[:, sl], T2[:, sl])
                    nc.vector.tensor_scalar(T4[:, sl], s4[:, sl], 2.0, -1.0,
                                            ALU.mult, ALU.add)
                    nc.vector.tensor_copy(b4[:, sl], T4[:, sl])

                    # Degrees 5..8 are leaves (no downstream consumer), so
                    # they can be produced directly in their matmul dtype:
                    #   T5 = 2 T2 T3 - T1, T7 = 2 T3 T4 - T1 from bf16
                    #   operands (bf16 DVE ops run in 2x mode)
                    #   T6 = 2 T3^2 - 1, T8 = 2 T4^2 - 1 as one tensor_scalar
                    #   (fp32 squares from ACT)
                    nc.vector.tensor_mul(u5[:, sl], b2[:, sl], b3[:, sl])
                    nc.vector.scalar_tensor_tensor(b5[:, sl], u5[:, sl], 2.0,
                                                   b1[:, sl],
                                                   ALU.mult, ALU.subtract)

                    nc.scalar.square(s6[:, sl], T3[:, sl])
                    nc.vector.tensor_scalar(b6[:, sl], s6[:, sl], 2.0, -1.0,
                                            ALU.mult, ALU.add)

                    nc.vector.tensor_mul(u7[:, sl], b3[:, sl], b4[:, sl])
                    nc.vector.scalar_tensor_tensor(b7[:, sl], u7[:, sl], 2.0,
                                                   b1[:, sl],
                                                   ALU.mult, ALU.subtract)

                    nc.scalar.square(s8[:, sl], T4[:, sl])
                    nc.vector.tensor_scalar(b8[:, sl], s8[:, sl], 2.0, -1.0,
                                            ALU.mult, ALU.add)

            # bias is only consumed at the end of each o-half pass; load it
            # late so it doesn't delay the xt/wt streams.
            bias_t = cpool.tile([P, O_DIM], f32, name="bias_t")
            nc.sync.dma_start(out=bias_t, in_=bias_d[:, :])

            # ---- contraction: two o-half passes over all K-chunks ----
            psums = [ppool.tile([P, ON], f32, tag=f"ps{b}", name=f"ps{b}")
                     for b in range(B_LOCAL // P)]
            # pass 0: o-half 0, all 8 batch banks (overlaps basis production)
            # pass 1a/1b: o-half 1 split in two bank halves, so the first
            # half's bias-adds + stores overlap the second half's matmuls
            # and the final tail only drains 4 banks.
            passes = [(0, 0, 8), (1, 0, 4), (1, 4, 8)]
            RS = 1.0 / W_SCALE

            # per-ic chunk schedules (production order)
            ic_sched = []
            for ic in range(IC):
                bf_prs = [pr for (i, pr) in BF_PAIRLIST if i == ic]
                ic_sched.append((ic, bf_prs, FP8_PAIRS[ic]))

            def bf_index(ic, pr):
                return BF_PAIRLIST.index((ic, pr))

            def f8_index(ic, pr):
                return F8_PAIRLIST.index((ic, pr))

            for pi, (oh, blo, bhi) in enumerate(passes):
                first = True
                for ic, bf_prs, prs in ic_sched:
                    # the fp8 W tile is consumed last in the ic block but
                    # fetched first, so the DoubleRow matmuls never wait
                    # on the stream.
                    wt8s = []
                    for pr in prs:
                        kf = f8_index(ic, pr)
                        wt8 = wpool.tile([P, 2, ON], fp8, tag="wt8",
                                         name=f"wt8_{pi}_{kf}")
                        nc.gpsimd.dma_start(out=wt8, in_=wf_d[oh, kf])
                        wt8s.append(wt8)
                    for bpr in bf_prs:
                        kb = bf_index(ic, bpr)
                        wt = wpool.tile([P, 2, ON], bf16, tag="wt",
                                        name=f"wt_{pi}_{kb}")
                        nc.gpsimd.dma_start(out=wt, in_=wb_d[oh, kb])
                        for j, d in enumerate(bpr):
                            bt = basis[(ic, d)]
                            for b in range(blo, bhi):
                                nc.tensor.matmul(
                                    psums[b],
                                    bt[:, b * P:(b + 1) * P],
                                    wt[:, j, :],
                                    start=first,
                                    stop=False,
                                )
                            first = False
                    for pr, wt8 in zip(prs, wt8s):
                        pt = basis8[(ic, pr)]
                        last = (ic == IC - 1 and pr == prs[-1])
                        for b in range(blo, bhi):
                            nc.tensor.matmul(
                                psums[b],
                                pt[:, :, b * P:(b + 1) * P],
                                wt8,
                                start=first,
                                stop=last,
                                perf_mode=DR,
                            )
                        first = False

                for b in range(blo, bhi):
                    ot = opool.tile([P, ON], f32, tag="ot", name=f"ot_{pi}_{b}")
                    bias_sl = bias_t[:, oh * ON:(oh + 1) * ON]
                    if pi == 0 and b < 4:
                        # banks 0-3 gate pass 1a: drain them via ACT copy so
                        # the start=True matmuls aren't stuck behind the
                        # serial DVE chain; rescale+bias in place later
                        # (overlaps the next pass).
                        nc.scalar.copy(ot, psums[b])
                        nc.vector.scalar_tensor_tensor(ot, ot, RS, bias_sl,
                                                       ALU.mult, ALU.add)
                        nc.sync.dma_start(
                            out=y_d[b * P:(b + 1) * P,
                                    oh * ON:(oh + 1) * ON],
                            in_=ot)
                    elif pi == len(passes) - 1:
                        # final pass: half-granularity rescale+bias, stores
                        # split across both DMA queues (the gpsimd queue is
                        # done with W by now). The EARLY banks go on gpsimd
                        # so its queue-drain (which gates the end-of-kernel
                        # barrier) finishes under the last sync stores.
                        for hh in range(2):
                            hsl = slice(hh * (ON // 2), (hh + 1) * (ON // 2))
                            nc.vector.scalar_tensor_tensor(
                                ot[:, hsl], psums[b][:, hsl], RS,
                                bias_sl[:, hsl], ALU.mult, ALU.add)
                            q = nc.gpsimd if b < 6 else nc.sync
                            q.dma_start(
                                out=y_d[b * P:(b + 1) * P,
                                        oh * ON + hh * (ON // 2):
                                        oh * ON + (hh + 1) * (ON // 2)],
                                in_=ot[:, hsl])
                    else:
                        nc.vector.scalar_tensor_tensor(ot, psums[b], RS,
                                                       bias_sl,
                                                       ALU.mult, ALU.add)
                        nc.sync.dma_start(
                            out=y_d[b * P:(b + 1) * P,
                                    oh * ON:(oh + 1) * ON],
                            in_=ot)
    nc.compile()  # bacc legalization: splits multi-sem waits (TRN2 allows 1)
    return nc


def _get_nc():
    global _nc
    if _nc is None:
        _nc = _build_nc()
    return _nc


def _prep_inputs(x, cheby_coeffs):
    x = np.asarray(x, dtype=np.float32)
    C = np.asarray(cheby_coeffs, dtype=np.float32)
    bf16 = ml_dtypes.bfloat16
    e4 = ml_dtypes.float8_e4m3

    Ws = C[:, :, 1:] * np.float32(W_SCALE)             # [I, O, 8] scaled
    Ws = Ws.reshape(IC, P, OH, ON, DEG)                # [ic, p, oh, on, d]

    # bf16 chunk pairs: Wb[oh, kb, p, 2, on]
    Wb = np.empty((OH, NBP, P, 2, ON), dtype=bf16)
    for kb, (ic, pr) in enumerate(BF_PAIRLIST):
        for si, d in enumerate(pr):
            Wb[:, kb, :, si, :] = np.transpose(Ws[ic, :, :, :, d - 1],
                                               (1, 0, 2)).astype(bf16)

    # fp8 pairs: Wf[oh, kf, p, 2, on]
    Wf = np.empty((OH, NF, P, 2, ON), dtype=e4)
    for kf, (ic, pr) in enumerate(F8_PAIRLIST):
        for si, d in enumerate(pr):
            w = np.clip(Ws[ic, :, :, :, d - 1], -W_CLIP, W_CLIP)
            Wf[:, kf, :, si, :] = np.transpose(w, (1, 0, 2)).astype(e4)

    bias = C[:, :, 0].sum(axis=0, dtype=np.float64).astype(np.float32)
    bias_rep = np.ascontiguousarray(np.broadcast_to(bias, (P, O_DIM)))

    in_maps = []
    for c in range(N_CORES):
        xt = np.ascontiguousarray(x[c * B_LOCAL:(c + 1) * B_LOCAL, :].T)
        in_maps.append({"xt": xt, "wb": Wb, "wf": Wf, "bias": bias_rep})
    return in_maps


def kernel(x, cheby_coeffs):
    global last_results
    nc = _get_nc()
    in_maps = _prep_inputs(x, cheby_coeffs)
    last_results = run_bass_kernel_spmd(nc, in_maps,
                                        core_ids=list(range(N_CORES)))
    y = np.concatenate([r["y"] for r in last_results.results], axis=0)
    return y


# revision 13
# speedup vs baseline: 1.1509x; 1.0166x over previous
"""ChebyKAN layer kernel for 8x Trainium2 NeuronCores.

Computes y[b,o] = sum_{i,d} T_d(tanh(x[b,i])) * C[i,o,d], d = 0..8,
with T_d the Chebyshev polynomials, via:
  - batch sharded 8 ways (1024 rows/core)
  - device computes T_1..T_8 with Chebyshev product identities
    (fp32 DVE/ACT)
  - d=0 term (T_0 == 1) folded into a host-precomputed bias[o]
  - mixed-precision contraction accumulating fp32 in PSUM:
      * most K-chunks as bf16 matmuls (128 K each)
      * FP8_PAIRS K-chunk pairs as fp8e4m3 DoubleRow matmuls (256 K
        per matmul at the same 512-cycle issue rate -> 2x throughput);
        the fp8 fraction is capped by the rel-err budget (2e-2)
  - all W pre-scaled by 2^18 on host so fp8 values use the e4m3 range;
    the output path rescales by 2^-18 and adds the bias in one DVE op
  - x is transposed on host so the basis is produced directly in
    [K, batch] (lhsT) layout; no on-device transpose needed.

Self-contained: hardcodes all shapes for inputs
  x: [8192, 1024] f32, cheby_coeffs: [1024, 1024, 9] f32.
"""

import numpy as np
import ml_dtypes

import concourse.bass as bass
import concourse.mybir as mybir
import concourse.tile as tile
from concourse import bacc
from concourse.bass_utils import run_bass_kernel_spmd

P = 128
B_TOTAL = 8192
I_DIM = 1024
O_DIM = 1024
DEG = 8              # degrees 1..8 on device (d=0 folded into bias)
N_CORES = 8
B_LOCAL = B_TOTAL // N_CORES     # 1024
IC = I_DIM // P                  # 8 input chunks
OH = 2                           # output halves (PSUM capacity: 8 banks)
ON = O_DIM // OH                 # 512

# fp8 DoubleRow assignment: per input-chunk, degree pairs computed in
# fp8e4m3 (both basis and W). Everything else is bf16.  22 of 64
# K-chunks in fp8 keeps max rel err ~0.018 (budget 2e-2).
FP8_PAIRS = {ic: [(7, 8)] for ic in range(IC)}
FP8_PAIRS[0] = [(5, 6), (7, 8)]
FP8_PAIRS[1] = [(5, 6), (7, 8)]
FP8_PAIRS[2] = [(5, 6), (7, 8)]

W_SCALE = float(2 ** 18)         # host multiplies W; device rescales
W_CLIP = 224.0                   # keep e4m3 (TRN: max 240) finite

_nc = None
last_results = None  # BassKernelResults of the most recent run (for profiling)


def _fp8_degs(ic):
    return {d for pr in FP8_PAIRS[ic] for d in pr}


def _chunk_lists():
    """bf16 chunk-pair list [(ic, (da, db))] and fp8 pair list
    [(ic, (dlo, dhi))], both in production order (ic-major,
    degree-minor). Every ic has an even number of bf16 degrees, so
    bf16 chunks pair up within an ic (one DMA per pair)."""
    bf, f8 = [], []
    for ic in range(IC):
        degs = _fp8_degs(ic)
        ds = [d for d in range(1, DEG + 1) if d not in degs]
        assert len(ds) % 2 == 0
        for j in range(0, len(ds), 2):
            bf.append((ic, (ds[j], ds[j + 1])))
        for pr in FP8_PAIRS[ic]:
            f8.append((ic, pr))
    return bf, f8


BF_PAIRLIST, F8_PAIRLIST = _chunk_lists()
NBP = len(BF_PAIRLIST)     # 21 with n=22
NF = len(F8_PAIRLIST)      # 11


def _ensure_ntff_hook():
    """bass_utils' trace path imports antenv.axon_hooks unconditionally, but
    this agent image's antenv package lacks that module. Synthesize it (with
    the real libaxon NTFF hook when available) so a BASS_TRACE=1 run traces
    instead of crashing."""
    import sys
    import types

    try:
        import antenv.axon_hooks  # noqa: F401
        return
    except ImportError:
        pass
    try:
        import antenv
    except ImportError:
        return
    hook = None
    try:
        from trn_agent_boot.trn_boot import _ntff_profile_via_ctypes
        hook = _ntff_profile_via_ctypes("/opt/axon/libaxon_pjrt.so")
    except Exception:
        hook = None
    mod = types.ModuleType("antenv.axon_hooks")
    state = {"hook": hook}
    mod.set_axon_ntff_profile_hook = lambda h: state.__setitem__("hook", h)
    mod.get_axon_ntff_profile_hook = lambda: state["hook"]
    sys.modules["antenv.axon_hooks"] = mod
    antenv.axon_hooks = mod


_ensure_ntff_hook()


def _build_nc():
    nc = bacc.Bacc()
    f32 = mybir.dt.float32
    bf16 = mybir.dt.bfloat16
    fp8 = mybir.dt.float8e4
    AF = mybir.ActivationFunctionType
    ALU = mybir.AluOpType
    DR = mybir.MatmulPerfMode.DoubleRow

    xt_d = nc.dram_tensor("xt", [I_DIM, B_LOCAL], f32, kind="ExternalInput")
    wb_d = nc.dram_tensor("wb", [OH, NBP, P, 2, ON], bf16, kind="ExternalInput")
    wf_d = nc.dram_tensor("wf", [OH, NF, P, 2, ON], fp8, kind="ExternalInput")
    bias_d = nc.dram_tensor("bias", [P, O_DIM], f32, kind="ExternalInput")
    y_d = nc.dram_tensor("y", [B_LOCAL, O_DIM], f32, kind="ExternalOutput")

    with tile.TileContext(nc) as tc:
        with (
            tc.tile_pool(name="const", bufs=1) as cpool,
            tc.tile_pool(name="xin", bufs=2) as xpool,
            tc.tile_pool(name="fwork", bufs=2) as fpool,
            tc.tile_pool(name="basis", bufs=1) as bpool,
            tc.tile_pool(name="wstream", bufs=12) as wpool,
            tc.tile_pool(name="outbuf", bufs=4) as opool,
            tc.tile_pool(name="acc", bufs=1, space="PSUM") as ppool,
        ):
            # ---- PE warm-up ----
            # HAM un-throttles the PE clock (1.2 -> 2.4 GHz) only after
            # ~3.4us of sustained matmul activity. Burn that window on dummy
            # matmuls into psum bank 0 while the first xt/wt DMAs are in
            # flight; the real k=0 matmul re-starts the bank (start=True).
            warm = cpool.tile([P, ON], bf16, name="warm")
            nc.vector.memset(warm, 1.0)
            warm_ps = ppool.tile([P, ON], f32, tag="ps0", name="warm_ps")
            for wi in range(3):
                nc.tensor.matmul(warm_ps, warm[:, 0:P], warm,
                                 start=(wi == 0), stop=(wi == 2))

            # ---- basis production: T_1..T_8 per 128-row chunk of i ----
            basis = {}       # (ic, d) -> bf16 tile [P, B_LOCAL]
            basis8 = {}      # (ic, (dlo, dhi)) -> fp8 pair tile [P, 2, B_LOCAL]

            for ic in range(IC):
                # ic == 0 runs every op on two half-tiles: the first real
                # matmuls can start one half-op earlier while the PE is
                # fresh off the warm-up.
                slices = ([slice(0, B_LOCAL // 2), slice(B_LOCAL // 2, B_LOCAL)]
                          if ic == 0 else [slice(0, B_LOCAL)])

                degs8 = _fp8_degs(ic)

                # xt on the HWDGE (sync) queue: issues in parallel with the
                # gpsimd wt stream and has lower first-byte latency.
                xt_t = xpool.tile([P, B_LOCAL], f32, tag="xt", name=f"xt_{ic}")
                for sl in slices:
                    nc.sync.dma_start(out=xt_t[:, sl],
                                      in_=xt_d[ic * P:(ic + 1) * P, sl])

                def btile(d):
                    bt = bpool.tile([P, B_LOCAL], bf16, tag=f"b_{ic}_{d}",
                                    name=f"b_{ic}_{d}")
                    basis[(ic, d)] = bt
                    return bt

                def bdst(d):
                    """Output AP for degree d: slot of an fp8 pair tile if
                    d is fp8, else a bf16 basis tile."""
                    if d in degs8:
                        pr = next(p for p in FP8_PAIRS[ic] if d in p)
                        pt = basis8.get((ic, pr))
                        if pt is None:
                            pt = bpool.tile([P, 2, B_LOCAL], fp8,
                                            tag=f"bp_{ic}_{pr[0]}",
                                            name=f"bp_{ic}_{pr[0]}")
                            basis8[(ic, pr)] = pt
                        return pt[:, pr.index(d), :]
                    return btile(d)

                # T1 = tanh(x) (no clip: the recurrence is stable for |t|<=1
                # and T_d(+-1) is finite; deviation from the reference's
                # clip at 0.999 is ~1e-6 on y)
                t = fpool.tile([P, B_LOCAL], f32, tag="T1", name=f"t_{ic}")
                s2 = fpool.tile([P, B_LOCAL], f32, tag="sq", name=f"s2_{ic}")
                T2 = fpool.tile([P, B_LOCAL], f32, tag="T2", name=f"T2_{ic}",
                                bufs=1)
                V3 = fpool.tile([P, B_LOCAL], f32, tag="u", name=f"V3_{ic}")
                T3 = fpool.tile([P, B_LOCAL], f32, tag="T3", name=f"T3_{ic}",
                                bufs=1)
                s4 = fpool.tile([P, B_LOCAL], f32, tag="sq", name=f"s4_{ic}")
                T4 = fpool.tile([P, B_LOCAL], f32, tag="T4", name=f"T4_{ic}",
                                bufs=1)
                u5 = fpool.tile([P, B_LOCAL], bf16, tag="ub", name=f"u5_{ic}")
                s6 = fpool.tile([P, B_LOCAL], f32, tag="sq", name=f"s6_{ic}")
                u7 = fpool.tile([P, B_LOCAL], bf16, tag="ub", name=f"u7_{ic}")
                s8 = fpool.tile([P, B_LOCAL], f32, tag="sq", name=f"s8_{ic}")
                b1 = bdst(1)
                b2 = bdst(2)
                b3 = bdst(3)
                b4 = bdst(4)
                b5 = bdst(5)
                b6 = bdst(6)
                b7 = bdst(7)
                b8 = bdst(8)

                for sl in slices:
                    nc.scalar.activation(t[:, sl], xt_t[:, sl], AF.Tanh)
                    # DVE cast: shortens the tanh -> first-matmul chain
                    nc.vector.tensor_copy(b1[:, sl], t[:, sl])

                    # T2 = 2 t^2 - 1
                    nc.scalar.square(s2[:, sl], t[:, sl])
                    nc.vector.tensor_scalar(T2[:, sl], s2[:, sl], 2.0, -1.0,
                                            ALU.mult, ALU.add)
                    nc.scalar.copy(b2[:, sl], T2[:, sl])

                    # T3 = 2 t T2 - t = t * (2 T2 - 1)
                    nc.vector.tensor_scalar(V3[:, sl], T2[:, sl], 2.0, -1.0,
                                            ALU.mult, ALU.add)
                    nc.vector.tensor_mul(T3[:, sl], t[:, sl], V3[:, sl])
                    # b3/b4 casts on DVE: the ACT engine is the busier one
                    # in the production phase
                    nc.vector.tensor_copy(b3[:, sl], T3[:, sl])

                    # T4 = 2 T2^2 - 1
                    nc.scalar.square(s4[:, sl], T2[:, sl])
                    nc.vector.tensor_scalar(T4[:, sl], s4[:, sl], 2.0, -1.0,
                                            ALU.mult, ALU.add)
                    nc.vector.tensor_copy(b4[:, sl], T4[:, sl])

                    # Degrees 5..8 are leaves (no downstream consumer), so
                    # they can be produced directly in their matmul dtype:
                    #   T5 = 2 T2 T3 - T1, T7 = 2 T3 T4 - T1 from bf16
                    #   operands (bf16 DVE ops run in 2x mode)
                    #   T6 = 2 T3^2 - 1, T8 = 2 T4^2 - 1 as one tensor_scalar
                    #   (fp32 squares from ACT)
                    nc.vector.tensor_mul(u5[:, sl], b2[:, sl], b3[:, sl])
                    nc.vector.scalar_tensor_tensor(b5[:, sl], u5[:, sl], 2.0,
                                                   b1[:, sl],
                                                   ALU.mult, ALU.subtract)

                    nc.scalar.square(s6[:, sl], T3[:, sl])
                    nc.vector.tensor_scalar(b6[:, sl], s6[:, sl], 2.0, -1.0,
                                            ALU.mult, ALU.add)

                    nc.vector.tensor_mul(u7[:, sl], b3[:, sl], b4[:, sl])
                    nc.vector.scalar_tensor_tensor(b7[:, sl], u7[:, sl], 2.0,
                                                   b1[:, sl],
                                                   ALU.mult, ALU.subtract)

                    nc.scalar.square(s8[:, sl], T4[:, sl])
                    nc.vector.tensor_scalar(b8[:, sl], s8[:, sl], 2.0, -1.0,
                                            ALU.mult, ALU.add)

            # bias is only consumed at the end of each o-half pass; load it
            # late so it doesn't delay the xt/wt streams.
            bias_t = cpool.tile([P, O_DIM], f32, name="bias_t")
            nc.sync.dma_start(out=bias_t, in_=bias_d[:, :])

            # ---- contraction: two o-half passes over all K-chunks ----
            psums = [ppool.tile([P, ON], f32, tag=f"ps{b}", name=f"ps{b}")
                     for b in range(B_LOCAL // P)]
            # pass 0: o-half 0, all 8 batch banks (overlaps basis production)
            # pass 1a/1b: o-half 1 split in two bank halves, so the first
            # half's bias-adds + stores overlap the second half's matmuls
            # and the final tail only drains 4 banks.
            passes = [(0, 0, 8), (1, 0, 4), (1, 4, 8)]
            RS = 1.0 / W_SCALE

            # per-ic chunk schedules (production order)
            ic_sched = []
            for ic in range(IC):
                bf_prs = [pr for (i, pr) in BF_PAIRLIST if i == ic]
                ic_sched.append((ic, bf_prs, FP8_PAIRS[ic]))

            def bf_index(ic, pr):
                return BF_PAIRLIST.index((ic, pr))

            def f8_index(ic, pr):
                return F8_PAIRLIST.index((ic, pr))

            # Group schedule: one entry per W tile. Pass 0 consumes each
            # ic's fp8 (DoubleRow) groups one ic-slot late, because the
            # fp8 basis degrees are the tail of the per-ic production
            # chain; passes over ready basis use plain production order.
            def group_order(delay_dr):
                nat = []          # (kind, ic, pr)
                for ic, bf_prs, prs in ic_sched:
                    nat.append([("bf", ic, pr) for pr in bf_prs]
                               + [("f8", ic, pr) for pr in prs])
                if not delay_dr:
                    return [g for blk in nat for g in blk]
                out = []
                pend = []        # delayed DR groups
                for blk in nat:
                    out.extend(g for g in blk if g[0] == "bf")
                    out.extend(pend)
                    pend = [g for g in blk if g[0] == "f8"]
                out.extend(pend)
                return out

            def issue_group(pi, oh, g):
                kind, ic, pr = g
                if kind == "bf":
                    kb = bf_index(ic, pr)
                    wt = wpool.tile([P, 2, ON], bf16, tag="wt",
                                    name=f"wt_{pi}_{kb}")
                    nc.gpsimd.dma_start(out=wt, in_=wb_d[oh, kb])
                else:
                    kf = f8_index(ic, pr)
                    wt = wpool.tile([P, 2, ON], fp8, tag="wt8",
                                    name=f"wt8_{pi}_{kf}")
                    nc.gpsimd.dma_start(out=wt, in_=wf_d[oh, kf])
                return wt

            def group_mms(g, wt, b, start, stop):
                kind, ic, pr = g
                if kind == "bf":
                    for j, d in enumerate(pr):
                        bt = basis[(ic, d)]
                        nc.tensor.matmul(
                            psums[b], bt[:, b * P:(b + 1) * P], wt[:, j, :],
                            start=start, stop=(stop and j == 1))
                        start = False
                else:
                    pt = basis8[(ic, pr)]
                    nc.tensor.matmul(
                        psums[b], pt[:, :, b * P:(b + 1) * P], wt,
                        start=start, stop=stop, perf_mode=DR)

            TAILG = 6  # trailing groups swept bank-major to stagger drains

            for pi, (oh, blo, bhi) in enumerate(passes):
                final = pi == len(passes) - 1
                groups = group_order(delay_dr=(pi == 0))
                head = groups[:-TAILG] if final else groups
                tails = groups[-TAILG:] if final else []
                tail_tiles = []
                for gi, g in enumerate(head):
                    wt = issue_group(pi, oh, g)
                    for b in range(blo, bhi):
                        group_mms(g, wt, b, start=(gi == 0),
                                  stop=(not final and gi == len(head) - 1))
                # final pass: the trailing groups are fetched up front and
                # swept bank-major, so banks finish staggered and each
                # bank's rescale+store overlaps the next bank's matmuls.
                # All stores ride the sync (HWDGE) queue: the gpsimd
                # (SWDGE) queue needs ~8us to drain after its last DMA and
                # would gate the end-of-kernel barrier.
                for g in tails:
                    tail_tiles.append(issue_group(pi, oh, g))
                for b in range(blo, bhi):
                    for gi, g in enumerate(tails):
                        group_mms(g, tail_tiles[gi], b, start=False,
                                  stop=(gi == len(tails) - 1))
                    if final:
                        ot = opool.tile([P, ON], f32, tag="ot",
                                        name=f"ot_{pi}_{b}")
                        bias_sl = bias_t[:, oh * ON:(oh + 1) * ON]
                        for hh in range(2):
                            hsl = slice(hh * (ON // 2), (hh + 1) * (ON // 2))
                            nc.vector.scalar_tensor_tensor(
                                ot[:, hsl], psums[b][:, hsl], RS,
                                bias_sl[:, hsl], ALU.mult, ALU.add)
                            nc.sync.dma_start(
                                out=y_d[b * P:(b + 1) * P,
                                        oh * ON + hh * (ON // 2):
                                        oh * ON + (hh + 1) * (ON // 2)],
                                in_=ot[:, hsl])

                if final:
                    continue
                for b in range(blo, bhi):
                    ot = opool.tile([P, ON], f32, tag="ot", name=f"ot_{pi}_{b}")
                    bias_sl = bias_t[:, oh * ON:(oh + 1) * ON]
                    if pi == 0 and b < 4:
                        # banks 0-3 gate pass 1a: drain them via ACT copy so
                        # the start=True matmuls aren't stuck behind the
                        # serial DVE chain; rescale+bias in place later
                        # (overlaps the next pass).
                        nc.scalar.copy(ot, psums[b])
                        nc.vector.scalar_tensor_tensor(ot, ot, RS, bias_sl,
                                                       ALU.mult, ALU.add)
                        nc.sync.dma_start(
                            out=y_d[b * P:(b + 1) * P,
                                    oh * ON:(oh + 1) * ON],
                            in_=ot)
                    else:
                        nc.vector.scalar_tensor_tensor(ot, psums[b], RS,
                                                       bias_sl,
                                                       ALU.mult, ALU.add)
                        nc.sync.dma_start(
                            out=y_d[b * P:(b + 1) * P,
                                    oh * ON:(oh + 1) * ON],
                            in_=ot)
    nc.compile()  # bacc legalization: splits multi-sem waits (TRN2 allows 1)
    return nc


def _get_nc():
    global _nc
    if _nc is None:
        _nc = _build_nc()
    return _nc


def _prep_inputs(x, cheby_coeffs):
    x = np.asarray(x, dtype=np.float32)
    C = np.asarray(cheby_coeffs, dtype=np.float32)
    bf16 = ml_dtypes.bfloat16
    e4 = ml_dtypes.float8_e4m3

    Ws = C[:, :, 1:] * np.float32(W_SCALE)             # [I, O, 8] scaled
    Ws = Ws.reshape(IC, P, OH, ON, DEG)                # [ic, p, oh, on, d]

    # bf16 chunk pairs: Wb[oh, kb, p, 2, on]
    Wb = np.empty((OH, NBP, P, 2, ON), dtype=bf16)
    for kb, (ic, pr) in enumerate(BF_PAIRLIST):
        for si, d in enumerate(pr):
            Wb[:, kb, :, si, :] = np.transpose(Ws[ic, :, :, :, d - 1],
                                               (1, 0, 2)).astype(bf16)

    # fp8 pairs: Wf[oh, kf, p, 2, on]
    Wf = np.empty((OH, NF, P, 2, ON), dtype=e4)
    for kf, (ic, pr) in enumerate(F8_PAIRLIST):
        for si, d in enumerate(pr):
            w = np.clip(Ws[ic, :, :, :, d - 1], -W_CLIP, W_CLIP)
            Wf[:, kf, :, si, :] = np.transpose(w, (1, 0, 2)).astype(e4)

    bias = C[:, :, 0].sum(axis=0, dtype=np.float64).astype(np.float32)
    bias_rep = np.ascontiguousarray(np.broadcast_to(bias, (P, O_DIM)))

    in_maps = []
    for c in range(N_CORES):
        xt = np.ascontiguousarray(x[c * B_LOCAL:(c + 1) * B_LOCAL, :].T)
        in_maps.append({"xt": xt, "wb": Wb, "wf": Wf, "bias": bias_rep})
    return in_maps


def kernel(x, cheby_coeffs):
    global last_results
    nc = _get_nc()
    in_maps = _prep_inputs(x, cheby_coeffs)
    last_results = run_bass_kernel_spmd(nc, in_maps,
                                        core_ids=list(range(N_CORES)))
    y = np.concatenate([r["y"] for r in last_results.results], axis=0)
    return y
